# revision 1
# baseline (speedup 1.0000x reference)
"""Sparse-attention Trainium2 kernel, 8-core SPMD.

Sharding: one head per NeuronCore (8 heads / 8 cores), batch replicated.
Each core computes, for its head h and all 4 batches:
  qkv proj -> RoPE -> S^T = K @ Q^T -> P^T = exp(S^T) * exp(bias^T)
  -> outT = V^T @ P^T (plus matmul row-sums) -> out-proj -> / rowsum
and writes a full-shape [4, 2048, 512] partial (its head's contribution
to the output projection). The host sums the 8 partials (the tensor
parallel all-reduce) in numpy.

Everything runs in fp16 on the TensorEngine with fp32 PSUM accumulation.
pos_bias is folded in as exp(S + b) = exp(S) * exp(b) with exp(b)
precomputed on host, so the bias add becomes a 2x-mode fp16 DVE multiply.
"""

import numpy as np

B, N, C = 4, 2048, 512
HEADS, D = 8, 64
NCORES = 8
ROPE_THETA = 10000.0

NT = N // 128       # 16 seq tiles of 128
IC = N // 512       # 4 i-chunks of 512
F16 = np.float16

_cache = {}


def _rope_tables():
    inv = 1.0 / (ROPE_THETA ** (np.arange(0, D, 2, dtype=np.float64) / D))
    freqs = np.arange(N, dtype=np.float64)[:, None] * inv            # [N, 32]
    freqs = np.repeat(freqs, 2, axis=-1)                             # [N, 64]
    cos = np.cos(freqs)
    sin = np.sin(freqs)
    # rotate_half: out[0::2] = -x[1::2]; out[1::2] = x[0::2]
    # q_ro = q*cos + swap(q)*sin_signed, swap = pairwise swap
    sin_signed = sin.copy()
    sin_signed[:, 0::2] *= -1.0
    scale = D ** -0.5
    return cos, sin_signed, scale


def _sb_tab(t):
    # [N, D] -> SBUF layout [128, NT*D]: tile nt at cols nt*D:(nt+1)*D
    return np.ascontiguousarray(
        t.reshape(NT, 128, D).transpose(1, 0, 2).reshape(128, NT * D).astype(F16))


def _build():
    import concourse.bacc as bacc
    import concourse.mybir as mybir
    import concourse.tile as tile

    exp_fn = mybir.ActivationFunctionType.Exp
    fp16 = mybir.dt.float16
    fp32 = mybir.dt.float32

    nc = bacc.Bacc(None)

    xT = nc.declare_dram_parameter("xT", [B, C, N], fp16, isOutput=False)
    wqkvT = nc.declare_dram_parameter("wqkvT", [C, 3 * D], fp16, isOutput=False)
    woT2 = nc.declare_dram_parameter("woT2", [128, C], fp16, isOutput=False)
    eb = nc.declare_dram_parameter("eb", [N, N], fp16, isOutput=False)
    out_ext = nc.declare_dram_parameter("out", [B, N, C], fp32, isOutput=True)

    cos, sin_signed, scale = _rope_tables()
    cosq_h = nc.inline_tensor(_sb_tab(cos * scale), name="cosq")
    sinq_h = nc.inline_tensor(_sb_tab(sin_signed * scale), name="sinq")
    cosk_h = nc.inline_tensor(_sb_tab(cos), name="cosk")
    sink_h = nc.inline_tensor(_sb_tab(sin_signed), name="sink")
    ident_h = nc.inline_tensor(np.eye(128, dtype=F16), name="ident")
    ones_h = nc.inline_tensor(np.ones((128, 1), dtype=F16), name="ones")

    with tile.TileContext(nc) as tc:
        with (
            tc.tile_pool(name="const", bufs=1) as cpool,
            tc.tile_pool(name="xt", bufs=8) as xtp,
            tc.tile_pool(name="qk", bufs=1) as qkp,
            tc.tile_pool(name="rope", bufs=3) as rpp,
            tc.tile_pool(name="ptp", bufs=4) as ptp,
            tc.tile_pool(name="ebp", bufs=3) as ebp,
            tc.tile_pool(name="outsb", bufs=4) as osb,
            tc.tile_pool(name="psA", bufs=2, space="PSUM") as psA,
            tc.tile_pool(name="psB", bufs=4, space="PSUM") as psB,
            tc.tile_pool(name="dramp", bufs=4, space="DRAM") as dramp,
        ):
            # ---- persistent SBUF tensors ----
            cosq = cpool.tile([128, NT * D], fp16, tag="cosq")
            sinq = cpool.tile([128, NT * D], fp16, tag="sinq")
            cosk = cpool.tile([128, NT * D], fp16, tag="cosk")
            sink = cpool.tile([128, NT * D], fp16, tag="sink")
            ident = cpool.tile([128, 128], fp16, tag="ident")
            ones = cpool.tile([128, 1], fp16, tag="ones")
            wq = cpool.tile([128, 4 * 3 * D], fp16, tag="wq")   # 4 c-chunks
            wo = cpool.tile([128, C], fp16, tag="wo")
            nc.gpsimd.dma_start(cosq[:], cosq_h[:])
            nc.gpsimd.dma_start(sinq[:], sinq_h[:])
            nc.gpsimd.dma_start(cosk[:], cosk_h[:])
            nc.gpsimd.dma_start(sink[:], sink_h[:])
            nc.gpsimd.dma_start(ident[:], ident_h[:])
            nc.gpsimd.dma_start(ones[:], ones_h[:])
            for cc in range(4):
                nc.gpsimd.dma_start(
                    wq[:, cc * 192:(cc + 1) * 192],
                    wqkvT[cc * 128:(cc + 1) * 128, :])
            nc.gpsimd.dma_start(wo[:], woT2[:])

            # qT/kT stacked per batch-pair: partitions 0:64 = batch even,
            # 64:128 = batch odd.  outT is per batch on partitions 0:64.
            qT = [qkp.tile([128, N], fp16, tag=f"qT{p}", name=f"qT{p}")
                  for p in range(2)]
            kT = [qkp.tile([128, N], fp16, tag=f"kT{p}", name=f"kT{p}")
                  for p in range(2)]
            vsb = [qkp.tile([128, NT * (D + 1)], fp16, tag=f"v{b}",
                            name=f"v{b}") for b in range(B)]
            for b in range(B):
                nc.gpsimd.memset(vsb[b][:], 1.0)
            outT = [qkp.tile([64, N], fp16, tag=f"outT{b}", name=f"outT{b}")
                    for b in range(B)]
            rs_r = qkp.tile([128, 4 * IC * 4], fp32, tag="rs")  # recip rowsums
            rs_raw = qkp.tile([128, 4 * IC * 4], fp32, tag="rsraw")

            # ---- phase A: qkv proj + rope + paired transposes ----
            # Batches processed in pairs: [tensor_b0 | tensor_b1] packs in
            # the free dim; one [128,128] PE transpose lands b0 at
            # partitions 0:64 and b1 at 64:128, so every DVE copy stays
            # partition-aligned (DVE lanes cannot shift partitions).
            for pr in range(2):
                bpair = (2 * pr, 2 * pr + 1)
                xt = [xtp.tile([128, N], fp16, tag="xt", name=f"xt{i}")
                      for i in range(8)]
                for i, b in enumerate(bpair):
                    for cc in range(4):
                        nc.sync.dma_start(
                            xt[4 * i + cc][:],
                            xT[b, cc * 128:(cc + 1) * 128, :])
                for nt in range(NT):
                    nsl = slice(nt * 128, (nt + 1) * 128)
                    ps_qkv = psA.tile([128, 1024], fp32, tag="psA",
                                      name="ps_qkv")
                    for i in range(2):
                        for cc in range(4):
                            nc.tensor.matmul(
                                ps_qkv[:, 512 * i:512 * i + 192],
                                xt[4 * i + cc][:, nsl],
                                wq[:, cc * 192:(cc + 1) * 192],
                                start=(cc == 0), stop=(cc == 3))
                    qkv = rpp.tile([128, 384], fp16, tag="qkv")
                    for i in range(2):
                        nc.vector.tensor_copy(
                            qkv[:, 192 * i:192 * i + 192],
                            ps_qkv[:, 512 * i:512 * i + 192])

                    dsl = slice(nt * D, (nt + 1) * D)
                    vdsl = slice(nt * (D + 1), nt * (D + 1) + D)
                    qpair = rpp.tile([128, 128], fp16, tag="qpair")
                    kpair = rpp.tile([128, 128], fp16, tag="kpair")
                    for i, b in enumerate(bpair):
                        nc.vector.tensor_copy(
                            vsb[b][:, vdsl],
                            qkv[:, 192 * i + 128:192 * i + 192])
                        for (o, ct, st, pair) in (
                                (192 * i, cosq, sinq, qpair),
                                (192 * i + 64, cosk, sink, kpair)):
                            src = qkv[:, o:o + 64]
                            t1 = pair[:, 64 * i:64 * i + 64]
                            sw = rpp.tile([128, 64], fp16, tag="sw")
                            sw_r = sw[:].rearrange("p (m two) -> p two m",
                                                   two=2)
                            src_r = src.rearrange("p (m two) -> p two m",
                                                  two=2)
                            nc.vector.tensor_copy(sw_r[:, 0, :], src_r[:, 1, :])
                            nc.vector.tensor_copy(sw_r[:, 1, :], src_r[:, 0, :])
                            nc.vector.tensor_mul(t1, src, ct[:, dsl])
                            nc.vector.tensor_mul(sw[:], sw[:], st[:, dsl])
                            nc.vector.tensor_add(t1, t1, sw[:])
                    for (pair, dst) in ((qpair, qT[pr]), (kpair, kT[pr])):
                        ps_t = psB.tile([128, 512], fp16, tag="psB")
                        nc.tensor.transpose(ps_t[:, 0:128], pair[:], ident[:])
                        nc.vector.tensor_copy(dst[:, nsl], ps_t[:, 0:128])

            # ---- phase B: attention ----
            # jt pairs: ps_s/pt are [128, 1024] = two j-tiles side by side.
            # PV lhsT = [v_jt | ones] (M=65): PSUM row 64 accumulates the
            # softmax denominators.  The [1,512] denominator row converts to
            # per-partition columns via a DRAM bounce with shape-trivial
            # DMAs (a [1,512] store, then four [128,1] column loads).
            for ic in range(IC):
                isl = slice(ic * 512, (ic + 1) * 512)
                ps_ov = [psB.tile([128, 512], fp32, tag="psB",
                                  name=f"ps_ov{b}") for b in range(B)]
                for jp in range(NT // 2):
                    ebt = ebp.tile([128, 1024], fp16, tag="eb")
                    for hh in range(2):
                        jt = 2 * jp + hh
                        nc.sync.dma_start(
                            ebt[:, hh * 512:(hh + 1) * 512],
                            eb[jt * 128:(jt + 1) * 128, isl])
                    for pr in range(2):
                        for bh in range(2):
                            b = 2 * pr + bh
                            po = 64 * bh
                            ps_s = psA.tile([128, 1024], fp32, tag="psA",
                                            name="ps_s")
                            for hh in range(2):
                                jt = 2 * jp + hh
                                jsl = slice(jt * 128, (jt + 1) * 128)
                                nc.tensor.matmul(
                                    ps_s[:, hh * 512:(hh + 1) * 512],
                                    kT[pr][po:po + 64, jsl],
                                    qT[pr][po:po + 64, isl],
                                    start=True, stop=True)
                            pt = ptp.tile([128, 1024], fp16, tag="pt")
                            nc.scalar.activation(pt[:], ps_s[:], func=exp_fn)
                            nc.vector.tensor_mul(pt[:], pt[:], ebt[:])
                            for hh in range(2):
                                jt = 2 * jp + hh
                                nc.tensor.matmul(
                                    ps_ov[b][0:65, :],
                                    vsb[b][:, jt * 65:jt * 65 + 65],
                                    pt[:, hh * 512:(hh + 1) * 512],
                                    start=(jp == 0 and hh == 0),
                                    stop=(jp == NT // 2 - 1 and hh == 1),
                                    skip_group_check=True)
                for b in range(B):
                    nc.vector.tensor_copy(outT[b][0:64, isl],
                                          ps_ov[b][0:64, :])
                    stage = osb.tile([128, C], fp32, tag="osb")
                    nc.vector.tensor_copy(stage[64:65, 0:512],
                                          ps_ov[b][64:65, :])
                    scr = dramp.tile([1, 512], fp32, tag="scr")
                    nc.sync.dma_start(scr[:], stage[64:65, 0:512])
                    for t in range(4):
                        c1 = ic * 16 + b * 4 + t
                        nc.gpsimd.dma_start(
                            rs_raw[:, c1:c1 + 1],
                            scr[0:1, t * 128:(t + 1) * 128].rearrange(
                                "o f -> f o"))
            nc.vector.reciprocal(rs_r[:], rs_raw[:])

            # ---- phase C: output projection + rowsum divide ----
            for b in range(B):
                for it in range(NT):
                    ic, t = it // 4, it % 4
                    ps_f = psB.tile([128, 512], fp32, tag="psB")
                    nc.tensor.matmul(
                        ps_f[:], outT[b][0:64, it * 128:(it + 1) * 128],
                        wo[0:64, :], start=True, stop=True)
                    osb_t = osb.tile([128, C], fp32, tag="osb")
                    c1 = ic * 16 + b * 4 + t
                    nc.vector.tensor_scalar_mul(
                        osb_t[:], ps_f[:], rs_r[:, c1:c1 + 1])
                    nc.sync.dma_start(
                        out_ext[b, it * 128:(it + 1) * 128, :], osb_t[:])
    nc.finalize()
    return nc


def kernel(x, pos_bias, w_qkv, w_out):
    from concourse.bass_utils import run_bass_kernel_spmd

    if "nc" not in _cache:
        _cache["nc"] = _build()
    nc = _cache["nc"]

    xT = np.ascontiguousarray(x.transpose(0, 2, 1)).astype(F16)
    in_maps = []
    for h in range(NCORES):
        hs = slice(h * D, (h + 1) * D)
        wq = np.concatenate(
            [w_qkv[hs], w_qkv[C + h * D:C + (h + 1) * D],
             w_qkv[2 * C + h * D:2 * C + (h + 1) * D]], axis=0)  # [192, C]
        wqkvT = np.ascontiguousarray(wq.T).astype(F16)           # [C, 192]
        woT = np.ascontiguousarray(w_out[:, hs].T).astype(F16)   # [64, C]
        woT2 = np.concatenate([woT, woT], axis=0)                # [128, C]
        ebm = np.exp(pos_bias[h].T.astype(np.float64)).astype(F16)
        in_maps.append({"xT": xT, "wqkvT": wqkvT, "woT2": woT2, "eb": ebm})

    _cache["in_maps"] = in_maps
    res = run_bass_kernel_spmd(nc, in_maps, core_ids=list(range(NCORES)))
    _cache["res"] = res
    out = np.zeros((B, N, C), np.float32)
    for i in range(NCORES):
        out += res.results[i]["out"]
    return out

